# revision 1
# baseline (speedup 1.0000x reference)
"""Self-contained Trainium2 (Bass) kernel for the BaseSigKernel problem.

kernel(xs, ys) -> (24, 24) float32 signature-kernel Gram matrix.

Math (per (x,y) pair; Salvi et al. finite-difference scheme, dyadic_order=1):
    a[r, s]   = <dy[r], dx[s]> / 4          (190x190, dyadic 2x2-duplicated)
    c1 = 1 + a/2 + a^2/12 ;  c2 = 1 - a^2/12
    u[0, :] = u[:, 0] = 1
    u[r+1, s+1] = (u[r+1, s] + u[r, s+1]) * c1[r, s] - u[r, s] * c2[r, s]
    result = u[190, 190]

Distribution: data-parallel over the batch_x axis - core ci owns b in
{3ci, 3ci+1, 3ci+2} x all 24 c's = 72 pairs, held on SBUF partitions
(three 32-partition bands; 24 used per band, the rest compute on zero
padding).

Per core, rows are processed serially; each row is ONE interleaved DVE
tensor_tensor_scan of length 380 alternating
    step 2s  : state = 1     * state + u_prev[s+1]
    step 2s+1: state = c1[s] * state + (-c2[s] * u_prev[s])
which reproduces the reference f32 association (u_left+u_up)*c1 - u_diag*c2
exactly. The scan's data1 is ubuf_prev[3:383] itself: u rows are stored
stride-2 (u[k] at ubuf[2k+1]) and one DVE multiply writes -c2*u into the
dead even lanes. Coefficient rows are produced just-in-time from tiny K=8
TensorE matmuls through ScalarE + GpSimd; one coefficient row serves two
PDE rows (dyadic duplication).

Measured DVE cost model (TRN2): scan = 153 + 2.08*L ns, tensor_tensor =
155 + 1.04*L ns, independent of partition count and stride. The 2-op row
(TT 190 + scan 380) minimizes fixed+marginal cost; 3-op splits with a
shorter scan lose on the ~155ns/instruction fixed overhead.

Both half-res coefficient streams land in one p12 tile and ONE ScalarE
Copy-broadcast expands them into cf's two odd-lane regions (uniform
stride-4 pattern over [1:760)); a single producer instruction means one
semaphore edge covers both DVE consumers (TT + scan) per coeff row
(-3% wall time vs separate c1/c2neg expansions). The expansion must be
a bit-exact Copy: computing c1 as (a/2 + a^2/12) + 1 instead of the
reference's (1 + a/2) + a^2/12 is a +-1-ulp change that amplifies
~1000x through the multiplicative recurrence and fails the near-zero
Gram entries (2.9e-2 vs 3.5e-3 max rel).
"""

import math
from contextlib import ExitStack

import numpy as np

import concourse.bacc as bacc
import concourse.mybir as mybir
import concourse.tile as tile
from concourse.ap import AP

F32 = mybir.dt.float32
Alu = mybir.AluOpType
Act = mybir.ActivationFunctionType

BX, BY, L, DIM = 24, 24, 96, 8
N_CORES = 8
BB = BX // N_CORES          # 3 b-values per core
BAND = 32                   # matmul output base partitions must be 0/32/64
P = BB * BAND               # 96 partitions; 24..31, 56..63, 88..95 are c-padding
NH = L - 1                  # 95: half-resolution grid length
NF = 2 * NH                 # 190: full-resolution grid length
INV_SQRT12 = 1.0 / math.sqrt(12.0)
CF_B = 380                  # coeff slot: [0:380) = [1|c1] interleaved, [380:760) = [x|c2neg] interleaved
UW = 2 * NF + 4             # u row buffer width (384): u[k] at ubuf[2k+1]


def _view(t_ap: AP, off: int, dims) -> AP:
    """Custom AP view of a tile: dims = [(step, count), ...] incl partition dim."""
    return AP(t_ap.tensor, t_ap.offset + off, [list(d) for d in dims])


def build_bass(ring: int = 4):
    nc = bacc.Bacc()
    # dyT and dxT packed into one tensor -> one DMA -> one PE sync wait
    inp_d = nc.declare_dram_parameter("inp", [DIM, NH * BAND + BB * NH], F32, isOutput=False)
    cf0_d = nc.declare_dram_parameter("cf0", [P, 4 * (CF_B + 2 * NF)], F32, isOutput=False)
    out_d = nc.declare_dram_parameter("out", [P, 1], F32, isOutput=True)

    with ExitStack() as ctx:
        tc = ctx.enter_context(tile.TileContext(nc))
        sbuf = ctx.enter_context(tc.tile_pool(name="sbuf", bufs=1))
        psum = ctx.enter_context(tc.tile_pool(name="psum", bufs=3, space="PSUM"))

        inp_t = sbuf.tile([DIM, NH * BAND + BB * NH], F32, name="inp_t", tag="inp_t")
        # HWDGE path (ScalarE queue): the GpSimd DIRECT path shows ~3-7us
        # completion latency that gates the first matmul
        nc.scalar.dma_start(inp_t[:], inp_d[:])

        # u rows, stride-2 storage: u[k] = ubuf[2k+1]; scan writes [2:382);
        # position 1 is the left boundary u[0] = 1 (preset, never written).
        ub = [sbuf.tile([P, UW], F32, name=f"u{i}", tag=f"u{i}") for i in range(2)]
        nc.vector.memset(ub[0][:], 1.0)   # row 0 = all ones
        nc.vector.memset(ub[1][:], 1.0)

        cfs = [
            sbuf.tile([P, CF_B + 2 * NF], F32, name=f"cf{i}", tag=f"cf{i}")
            for i in range(ring)
        ]
        t2s = [
            sbuf.tile([P, NH], F32, name=f"t2{i}", tag=f"t2{i}") for i in range(ring)
        ]
        # p12: [c1 (95) | c2neg (95)] half-res, bit-exact baseline values;
        # ONE Copy-broadcast expansion writes both cf regions (single DVE edge)
        p12s = [
            sbuf.tile([P, 192], F32, name=f"p12_{i}", tag=f"p12_{i}")
            for i in range(ring)
        ]
        s12s = [
            sbuf.tile([P, NH], F32, name=f"s12{i}", tag=f"s12{i}")
            for i in range(ring)
        ]

        # coeff rows 0-3 (incl. all even-lane 1.0s) are host-precomputed and
        # DMA'd straight into slots 0-3 (one pipelined DMA per slot so slot
        # 0's completion, which gates the first scan, doesn't slip), skipping
        # their matmul/scalar chains and the production-pipeline warm-up;
        # later ring users only rewrite odd lanes
        W = CF_B + 2 * NF
        for i in range(4):
            nc.scalar.dma_start(cfs[i][:], cf0_d[:, i * W : (i + 1) * W])

        # per-partition bias constant -1.0 for ACT
        cbias = sbuf.tile([P, 1], F32, name="cbias", tag="cbias")
        nc.gpsimd.memset(cbias[:], -1.0)

        def bcast_h(t_ap):
            # [P, NH] -> [P, NH, 2] with the last dim broadcast (step 0)
            p_step, p_cnt = t_ap.ap[0]
            return _view(t_ap, 0, [(p_step, p_cnt), (1, NH), (0, 2)])

        def produce_coeff(q):
            """One half-resolution coefficient row; serves PDE rows 2q, 2q+1."""
            if q < 4:
                return   # host-precomputed, DMA'd into slots 0-3
            pa_full = psum.tile([P, 512], F32, name="pa", tag="pa")
            pa = pa_full[:, 0:NH]
            lhsT = inp_t[:, q * BAND : (q + 1) * BAND]     # [8, 32] (24 real + 8 pad)
            for b in range(BB):
                nc.tensor.matmul(
                    pa[b * BAND : (b + 1) * BAND, :],
                    lhsT,
                    inp_t[:, NH * BAND + b * NH : NH * BAND + (b + 1) * NH],
                )
            cf, t2, s12 = cfs[q % ring], t2s[q % ring], s12s[q % ring]
            p12 = p12s[q % ring]
            cp_step, _ = cf.ap[0]
            pp_step, _ = p12.ap[0]
            # s12 = (a * 1/sqrt(12))^2 = a^2/12
            nc.scalar.activation(s12[:], pa[:], Act.Square, scale=INV_SQRT12)
            # t2 = 0.5*a + 1
            nc.scalar.activation(t2[:], pa[:], Act.Identity, bias=1.0, scale=0.5)
            # p12[95:190] = c2neg = s12 - 1 (same per-element math as before)
            nc.scalar.activation(
                p12[:, NH : 2 * NH], s12[:], Act.Identity, bias=cbias[:]
            )
            # p12[0:95] = c1 = t2 + s12, half-res on GpSimd (short op: the
            # Pool<->DVE shared SBUF port contends with the scans)
            nc.gpsimd.tensor_tensor(p12[:, 0:NH], t2[:], s12[:], Alu.add)
            # ONE Copy-broadcast expansion writes c1 into the odd lanes of
            # cf[0:380) and c2neg into the odd lanes of cf[380:760) (uniform
            # stride-4 pattern). Bit-exact passthrough; single producer ->
            # single DVE semaphore edge for both the TT and the scan.
            cf_w = _view(cf, 1, [(cp_step, P), (4, 2 * NH), (2, 2)])
            p12_r = _view(p12, 0, [(pp_step, P), (1, 2 * NH), (0, 2)])
            nc.scalar.activation(cf_w, p12_r, Act.Copy)

        def consume_row(r):
            cf = cfs[(r // 2) % ring]
            up = ub[r % 2]
            un = ub[(r + 1) % 2]
            u_step, _ = up.ap[0]
            if r == 0:
                # u_up == 1: the products are c2neg itself; read data1 straight
                # from the cf tile and skip the TT entirely
                nc.vector.tensor_tensor_scan(
                    un[:, 2 : 2 + 2 * NF],
                    cf[:, 0 : 2 * NF],
                    cf[:, CF_B : CF_B + 2 * NF],
                    1.0,
                    Alu.mult,
                    Alu.add,
                )
                return
            # write c2neg[s]*u_prev[s] into the DEAD even lanes of ubuf_prev
            # (they hold last row's scan intermediates), so that
            # ubuf_prev[3:383] is exactly the interleaved scan data1:
            #   t=2s   -> ubuf[3+2s] = u_prev[s+1]
            #   t=2s+1 -> ubuf[4+2s] = c2neg[s]*u_prev[s]
            cp_step, _ = cf.ap[0]
            nc.vector.tensor_tensor(
                _view(up, 4, [(u_step, P), (2, NF)]),
                _view(cf, CF_B + 1, [(cp_step, P), (2, NF)]),
                _view(up, 1, [(u_step, P), (2, NF)]),
                Alu.mult,
            )
            # interleaved scan: state=(d0*state)+d1 over 380 steps
            nc.vector.tensor_tensor_scan(
                un[:, 2 : 2 + 2 * NF],
                cf[:, 0 : 2 * NF],
                up[:, 3 : 3 + 2 * NF],
                1.0,
                Alu.mult,
                Alu.add,
            )

        # interleave production (lookahead AH slots) with consumption so
        # trace order matches dataflow.
        AH = ring - 2
        for q in range(AH):
            produce_coeff(q)
        for r in range(NF):
            if r % 2 == 0 and r // 2 + AH < NH:
                produce_coeff(r // 2 + AH)
            consume_row(r)

        # HWDGE path (SP/sync queue): the GpSimd DIRECT path's ~7us
        # completion latency sat on the exit barrier
        nc.sync.dma_start(out_d[:], ub[NF % 2][:, 2 * NF + 1 : 2 * NF + 2])

    nc.compile()
    return nc


def pack_inputs(xs: np.ndarray, ys: np.ndarray):
    """Full inputs -> per-core in_maps for run_bass_kernel_spmd."""
    xs = np.asarray(xs, np.float32)
    ys = np.asarray(ys, np.float32)
    dx = np.diff(xs, axis=1) * 0.5            # (24, 95, 8)
    dy = np.diff(ys, axis=1) * 0.5            # (24, 95, 8)
    dyT = np.zeros((DIM, NH, BAND), np.float32)
    dyT[:, :, :BY] = dy.transpose(2, 1, 0)
    dyT = dyT.reshape(DIM, NH * BAND)
    inv = np.float32(1.0 / math.sqrt(12.0))
    in_maps = []
    for ci in range(N_CORES):
        dxc = dx[ci * BB : (ci + 1) * BB]     # (3, 95, 8)
        dxT = dxc.transpose(2, 0, 1).reshape(DIM, BB * NH)
        inp = np.ascontiguousarray(np.concatenate([dyT, dxT], axis=1))
        # host-precomputed coeff row 0 (replicates the device fp32 math;
        # row-0-only host-vs-PE matmul noise is ~1 ulp and non-systematic)
        W = CF_B + 2 * NF
        cf0 = np.ones((P, 4 * W), np.float32)
        rep = np.repeat(np.arange(NH), 2)
        for q in range(4):
            a0 = np.zeros((P, NH), np.float32)
            for b in range(BB):
                a0[b * BAND : b * BAND + BY] = np.einsum(
                    "cd,jd->cj", dy[:, q, :], dxc[b], dtype=np.float32
                ).astype(np.float32)
            s12 = (a0 * inv) ** 2
            c1 = (np.float32(0.5) * a0 + np.float32(1.0)) + s12
            c2n = s12 - np.float32(1.0)
            cf0[:, q * W + 1 : q * W + 380 : 2] = c1[:, rep]
            cf0[:, q * W + CF_B + 1 : q * W + CF_B + 380 : 2] = c2n[:, rep]
        in_maps.append({"inp": inp, "cf0": cf0})
    return in_maps


def unpack_outputs(results) -> np.ndarray:
    """Per-core (96,1) outputs -> full (24,24)."""
    out = np.zeros((BX, BY), np.float32)
    for ci in range(N_CORES):
        res = np.asarray(results[ci]["out"]).reshape(P)
        for b in range(BB):
            out[ci * BB + b, :] = res[b * BAND : b * BAND + BY]
    return out


_NC_CACHE = None


def kernel(xs: np.ndarray, ys: np.ndarray) -> np.ndarray:
    """Full (24,96,8) inputs -> full (24,24) output, computed on 8 trn2 cores."""
    global _NC_CACHE
    from concourse.bass_utils import run_bass_kernel_spmd

    if _NC_CACHE is None:
        _NC_CACHE = build_bass()
    in_maps = pack_inputs(xs, ys)
    r = run_bass_kernel_spmd(_NC_CACHE, in_maps, list(range(N_CORES)))
    return unpack_outputs(r.results)



# revision 5
# speedup vs baseline: 1.0224x; 1.0224x over previous
"""Self-contained Trainium2 (Bass) kernel for the BaseSigKernel problem.

kernel(xs, ys) -> (24, 24) float32 signature-kernel Gram matrix.

Math (per (x,y) pair; Salvi et al. finite-difference scheme, dyadic_order=1):
    a[r, s]   = <dy[r], dx[s]> / 4          (190x190, dyadic 2x2-duplicated)
    c1 = 1 + a/2 + a^2/12 ;  c2 = 1 - a^2/12
    u[0, :] = u[:, 0] = 1
    u[r+1, s+1] = (u[r+1, s] + u[r, s+1]) * c1[r, s] - u[r, s] * c2[r, s]
    result = u[190, 190]

Distribution: data-parallel over the batch_x axis - core ci owns b in
{3ci, 3ci+1, 3ci+2} x all 24 c's = 72 pairs, held on SBUF partitions
(three 32-partition bands; 24 used per band, the rest compute on zero
padding).

Per core, rows are processed serially; each row is ONE interleaved DVE
tensor_tensor_scan of length 380 alternating
    step 2s  : state = 1     * state + u_prev[s+1]
    step 2s+1: state = c1[s] * state + (-c2[s] * u_prev[s])
which reproduces the reference f32 association (u_left+u_up)*c1 - u_diag*c2
exactly. The scan's data1 is ubuf_prev[3:383] itself: u rows are stored
stride-2 (u[k] at ubuf[2k+1]) and one DVE multiply writes -c2*u into the
dead even lanes. Any reassociation of the per-cell math (e.g. folding the
-c2*u product into scan multipliers via c1/c2 ratios) amplifies ~1000x
through the recurrence and fails the near-zero Gram entries; the exact
association - and hence the per-row TT - is forced.

Measured DVE cost model (TRN2): scan = 153 + 2.08*L ns, tensor_tensor =
155 + 1.04*L ns, independent of partition count and stride. The 2-op row
(TT 190 + scan 380) minimizes fixed+marginal cost; DVE floor is
190*(945+356) = 247us and everything else here is overhead-shaving:

- Coefficients are produced in GROUPS of 8 half-res rows (16 PDE rows)
  with exactly two consumer-visible producer instructions per group (one
  GpSimd even-lane memset + one ScalarE odd-lane Copy-broadcast), so the
  Vector sequencer executes ~2 semaphore waits per 16 rows instead of
  ~2 per 2 rows (each satisfied wait still costs ~68ns of sequencer time).
- Host precomputes group 0 (slots 0-7) in final interleaved form; the
  slot-0 DMA is issued first on the idle SP HWDGE queue so the first scan
  starts ~8.7us instead of ~12.6us (descriptor generation on the Scalar
  queue serializes behind ACT_TABLE_LOAD).
- ub memsets run on GpSimd so the Vector queue's first instruction is the
  row-0 scan.
- The output column (one f32 per partition) is transposed on the idle PE
  via an identity matmul to a contiguous [1, 96] PSUM row before the exit
  DMA: a [96,1] SBUF->DRAM DMA emits 96 4-byte descriptors (~6.8us); the
  transposed form is one descriptor.
"""

import math
from contextlib import ExitStack

import numpy as np

import concourse.bacc as bacc
import concourse.mybir as mybir
import concourse.tile as tile
from concourse.ap import AP

F32 = mybir.dt.float32
Alu = mybir.AluOpType
Act = mybir.ActivationFunctionType

BX, BY, L, DIM = 24, 24, 96, 8
N_CORES = 8
BB = BX // N_CORES          # 3 b-values per core
BAND = 32                   # matmul output base partitions must be 0/32/64
P = BB * BAND               # 96 partitions; 24..31, 56..63, 88..95 are c-padding
NH = L - 1                  # 95: half-resolution grid length
NF = 2 * NH                 # 190: full-resolution grid length
INV_SQRT12 = 1.0 / math.sqrt(12.0)
CF_B = 380                  # coeff slot: [0:380) = [1|c1] interleaved, [380:760) = [x|c2neg] interleaved
W = CF_B + 2 * NF           # 760: coeff slot width
UW = 2 * NF + 4             # u row buffer width (384): u[k] at ubuf[2k+1]
GS = 8                      # coeff slots per production group
RPG = 2 * GS                # 16 PDE rows per group
NG = (NH + GS - 1) // GS    # 12 groups (last has 7 slots)
RING = 3                    # cf group ring


def _view(t_ap: AP, off: int, dims) -> AP:
    """Custom AP view of a tile: dims = [(step, count), ...] incl partition dim."""
    return AP(t_ap.tensor, t_ap.offset + off, [list(d) for d in dims])


def build_bass():
    nc = bacc.Bacc()
    # dyT and dxT packed into one tensor -> one DMA -> one PE sync wait
    inp_d = nc.declare_dram_parameter("inp", [DIM, NH * BAND + BB * NH], F32, isOutput=False)
    cf0_d = nc.declare_dram_parameter("cf0", [P, GS * W], F32, isOutput=False)
    idn_d = nc.declare_dram_parameter("idn", [P, P], F32, isOutput=False)
    out_d = nc.declare_dram_parameter("out", [1, P], F32, isOutput=True)

    with ExitStack() as ctx:
        tc = ctx.enter_context(tile.TileContext(nc))
        sbuf = ctx.enter_context(tc.tile_pool(name="sbuf", bufs=1))
        psum = ctx.enter_context(tc.tile_pool(name="psum", bufs=2, space="PSUM"))
        psum1 = ctx.enter_context(tc.tile_pool(name="psum1", bufs=1, space="PSUM"))

        cfg = [
            sbuf.tile([P, GS * W], F32, name=f"cfg{i}", tag=f"cfg{i}")
            for i in range(RING)
        ]
        inp_t = sbuf.tile([DIM, NH * BAND + BB * NH], F32, name="inp_t", tag="inp_t")
        idn_t = sbuf.tile([P, P], F32, name="idn_t", tag="idn_t")
        ub = [sbuf.tile([P, UW], F32, name=f"u{i}", tag=f"u{i}") for i in range(2)]
        p12g = [
            sbuf.tile([P, GS * 2 * NH], F32, name=f"p12_{i}", tag=f"p12_{i}")
            for i in range(RING)
        ]
        s12g = [
            sbuf.tile([P, GS * NH], F32, name=f"s12{i}", tag=f"s12{i}")
            for i in range(RING)
        ]
        t2g = [
            sbuf.tile([P, GS * NH], F32, name=f"t2{i}", tag=f"t2{i}")
            for i in range(RING)
        ]
        cbias = sbuf.tile([P, 1], F32, name="cbias", tag="cbias")

        # Group 0 (slots 0-7) host-precomputed in final interleaved form.
        # Slot 0 rides its own DMA on the idle SP HWDGE queue so the first
        # scan's gate is the smallest possible transfer; the Scalar queue
        # carries inp/idn whose consumers have >5us of slack.
        nc.sync.dma_start(cfg[0][:, 0:W], cf0_d[:, 0:W])
        nc.sync.dma_start(cfg[0][:, W : 4 * W], cf0_d[:, W : 4 * W])
        nc.sync.dma_start(cfg[0][:, 4 * W : GS * W], cf0_d[:, 4 * W : GS * W])
        nc.scalar.dma_start(inp_t[:], inp_d[:])
        nc.scalar.dma_start(idn_t[:], idn_d[:])

        # ub presets on GpSimd: Vector's first instruction is the row-0 scan
        nc.gpsimd.memset(ub[0][:], 1.0)
        nc.gpsimd.memset(ub[1][:], 1.0)
        nc.gpsimd.memset(cbias[:], -1.0)

        def produce_group(g):
            """Slots [8g, 8g+ns) -> cfg[g%RING]; two consumer-visible producers."""
            gi = g % RING
            q0 = g * GS
            ns = min(GS, NH - q0)
            cfgt, p12, s12, t2 = cfg[gi], p12g[gi], s12g[gi], t2g[gi]
            pas = []
            for half in range((ns + 3) // 4):
                lo = half * 4
                hi = min(ns, lo + 4)
                pa_full = psum.tile([P, 512], F32, name=f"pa{g}_{half}", tag=f"pa{half}")
                pas.append(pa_full)
                for j in range(lo, hi):
                    q = q0 + j
                    lhsT = inp_t[:, q * BAND : (q + 1) * BAND]   # [8, 32]
                    for b in range(BB):
                        nc.tensor.matmul(
                            pa_full[b * BAND : (b + 1) * BAND, (j - lo) * NH : (j - lo + 1) * NH],
                            lhsT,
                            inp_t[:, NH * BAND + b * NH : NH * BAND + (b + 1) * NH],
                        )
            for j in range(ns):
                pa = pas[j // 4][:, (j % 4) * NH : (j % 4 + 1) * NH]
                sl = s12[:, j * NH : (j + 1) * NH]
                tl = t2[:, j * NH : (j + 1) * NH]
                # s12 = (a * 1/sqrt(12))^2 = a^2/12
                nc.scalar.activation(sl, pa, Act.Square, scale=INV_SQRT12)
                # t2 = 0.5*a + 1
                nc.scalar.activation(tl, pa, Act.Identity, bias=1.0, scale=0.5)
                # p12 slot layout: [c1h (95) | c2negh (95)]
                nc.scalar.activation(
                    p12[:, j * 2 * NH + NH : (j + 1) * 2 * NH], sl, Act.Identity, bias=cbias[:]
                )
                nc.gpsimd.tensor_tensor(
                    p12[:, j * 2 * NH : j * 2 * NH + NH], tl, sl, Alu.add
                )
            cstep, _ = cfgt.ap[0]
            pstep, _ = p12.ap[0]
            # even lanes (the scan's "1" multipliers) for the whole group
            nc.gpsimd.memset(_view(cfgt, 0, [(cstep, P), (2, ns * CF_B)]), 1.0)
            # odd lanes: the stride-4 pattern runs across slot boundaries, so
            # ONE Copy-broadcast expands all ns slots' c1+c2neg regions.
            nc.scalar.activation(
                _view(cfgt, 1, [(cstep, P), (4, ns * 2 * NH), (2, 2)]),
                _view(p12, 0, [(pstep, P), (1, ns * 2 * NH), (0, 2)]),
                Act.Copy,
            )

        def consume_row(r):
            cfgt = cfg[(r // RPG) % RING]
            off = ((r // 2) % GS) * W
            up = ub[r % 2]
            un = ub[(r + 1) % 2]
            u_step, _ = up.ap[0]
            c_step, _ = cfgt.ap[0]
            if r == 0:
                # u_up == 1: the products are c2neg itself; read data1 straight
                # from the cf slot and skip the TT entirely
                nc.vector.tensor_tensor_scan(
                    un[:, 2 : 2 + 2 * NF],
                    cfgt[:, off : off + 2 * NF],
                    cfgt[:, off + CF_B : off + CF_B + 2 * NF],
                    1.0,
                    Alu.mult,
                    Alu.add,
                )
                return
            # write c2neg[s]*u_prev[s] into the DEAD even lanes of ubuf_prev
            # (they hold last row's scan intermediates), so that
            # ubuf_prev[3:383] is exactly the interleaved scan data1:
            #   t=2s   -> ubuf[3+2s] = u_prev[s+1]
            #   t=2s+1 -> ubuf[4+2s] = c2neg[s]*u_prev[s]
            nc.vector.tensor_tensor(
                _view(up, 4, [(u_step, P), (2, NF)]),
                _view(cfgt, off + CF_B + 1, [(c_step, P), (2, NF)]),
                _view(up, 1, [(u_step, P), (2, NF)]),
                Alu.mult,
            )
            # interleaved scan: state=(d0*state)+d1 over 380 steps
            nc.vector.tensor_tensor_scan(
                un[:, 2 : 2 + 2 * NF],
                cfgt[:, off : off + 2 * NF],
                up[:, 3 : 3 + 2 * NF],
                1.0,
                Alu.mult,
                Alu.add,
            )

        # device production starts at group 1; 2-group lookahead
        produce_group(1)
        produce_group(2)
        for r in range(NF):
            if r % RPG == 0 and RPG <= r <= (NG - 3) * RPG:
                produce_group(r // RPG + 2)
            consume_row(r)

        # transpose the per-partition result column to a contiguous [1, P]
        # PSUM row on the idle PE, bounce through SBUF (DMA cannot read
        # PSUM), then one single-descriptor DMA out
        pout = psum1.tile([BAND, 512], F32, name="pout", tag="pout")
        orow = sbuf.tile([1, P], F32, name="orow", tag="orow")
        nc.tensor.matmul(
            pout[0:1, 0:P], ub[NF % 2][:, 2 * NF + 1 : 2 * NF + 2], idn_t[:, 0:P]
        )
        nc.scalar.activation(orow[0:1, 0:P], pout[0:1, 0:P], Act.Copy)
        nc.sync.dma_start(out_d[:], orow[0:1, 0:P])

    nc.compile()
    return nc


def pack_inputs(xs: np.ndarray, ys: np.ndarray):
    """Full inputs -> per-core in_maps for run_bass_kernel_spmd."""
    xs = np.asarray(xs, np.float32)
    ys = np.asarray(ys, np.float32)
    dx = np.diff(xs, axis=1) * 0.5            # (24, 95, 8)
    dy = np.diff(ys, axis=1) * 0.5            # (24, 95, 8)
    dyT = np.zeros((DIM, NH, BAND), np.float32)
    dyT[:, :, :BY] = dy.transpose(2, 1, 0)
    dyT = dyT.reshape(DIM, NH * BAND)
    inv = np.float32(1.0 / math.sqrt(12.0))
    idn = np.eye(P, dtype=np.float32)
    in_maps = []
    for ci in range(N_CORES):
        dxc = dx[ci * BB : (ci + 1) * BB]     # (3, 95, 8)
        dxT = dxc.transpose(2, 0, 1).reshape(DIM, BB * NH)
        inp = np.ascontiguousarray(np.concatenate([dyT, dxT], axis=1))
        # host-precomputed coeff group 0 (slots 0-7; replicates the device
        # fp32 math - host-vs-PE matmul noise is ~1 ulp and non-systematic)
        cf0 = np.ones((P, GS * W), np.float32)
        rep = np.repeat(np.arange(NH), 2)
        for q in range(GS):
            a0 = np.zeros((P, NH), np.float32)
            for b in range(BB):
                a0[b * BAND : b * BAND + BY] = np.einsum(
                    "cd,jd->cj", dy[:, q, :], dxc[b], dtype=np.float32
                ).astype(np.float32)
            s12 = (a0 * inv) ** 2
            c1 = (np.float32(0.5) * a0 + np.float32(1.0)) + s12
            c2n = s12 - np.float32(1.0)
            cf0[:, q * W + 1 : q * W + 380 : 2] = c1[:, rep]
            cf0[:, q * W + CF_B + 1 : q * W + CF_B + 380 : 2] = c2n[:, rep]
        in_maps.append({"inp": inp, "cf0": cf0, "idn": idn})
    return in_maps


def unpack_outputs(results) -> np.ndarray:
    """Per-core (1,96) outputs -> full (24,24)."""
    out = np.zeros((BX, BY), np.float32)
    for ci in range(N_CORES):
        res = np.asarray(results[ci]["out"]).reshape(P)
        for b in range(BB):
            out[ci * BB + b, :] = res[b * BAND : b * BAND + BY]
    return out


_NC_CACHE = None


def kernel(xs: np.ndarray, ys: np.ndarray) -> np.ndarray:
    """Full (24,96,8) inputs -> full (24,24) output, computed on 8 trn2 cores."""
    global _NC_CACHE
    from concourse.bass_utils import run_bass_kernel_spmd

    if _NC_CACHE is None:
        _NC_CACHE = build_bass()
    in_maps = pack_inputs(xs, ys)
    r = run_bass_kernel_spmd(_NC_CACHE, in_maps, list(range(N_CORES)))
    return unpack_outputs(r.results)
